# revision 1
# baseline (speedup 1.0000x reference)
"""Trainium2 Bass kernel for nn_Discriminator_87660282511381.

Data-parallel over 8 NeuronCores (64 samples each). Key algebraic
reductions baked in:
  - maxpool2x2 of the outer-sum s[b,j,c]+t[b,i,c] factorizes into
    per-axis pair-maxes (max of a sum of independent terms).
  - conv2 on the rank-1 (outer-sum) pooled map separates into two 1-D
    convs: out2[b,o,y,x] = U[b,o,y] + V[b,o,x]  (+b2, which cancels in BN).
  - BN2 batch stats reduce to 5 per-channel scalars -> 800B AllReduce.
  - relu(maxpool(bn2(out2))) = relu(scale*(maxU[b,o,i]+maxV[b,o,j])+shift)
    by monotonicity (g2>0); fc1 and fc2 (no nonlinearity between) collapse
    into a single weighted sum over the [40,11,11] grid.

The big outer-add G_raw = maxU (+) maxV is computed before the stats
AllReduce; only a fused scale/shift + relu + weighted reduce remains after
it, minimizing the exposed collective latency.
"""

import numpy as np
import ml_dtypes

import jax
import concourse.bacc as bacc
import concourse.mybir as mybir
import concourse.tile as tile_mod
from concourse.tile import TileContext

N_CORES = 8
B, L, E = 512, 50, 512
VOCAB = 32000
LP = L // 2                # 25 pooled positions
YD = LP - 2                # 23 = conv2 output length
PP = (YD - 1) // 2         # 11 pooled-2 positions
EPS = 1e-5
F32 = mybir.dt.float32
BF16 = mybir.dt.bfloat16
I16 = mybir.dt.int16

_CACHE = {}
LAST_EXEC_NS = None


def _patched_drain_and_barrier(self, tick_clock, wait_clock):
    # This walrus build rejects >1 sync-wait on Drain-class instructions;
    # fan the tail waits out one-per-NOP on the sync engine instead.
    nop = self.nc.sync.nop(nofuse=True, hint="tile_tail_wait")
    wait_clock.add_sem_waits(
        nop.ins, tile_mod.ScopedClock({None: tick_clock.global_clock})
    )
    waits = list(nop.ins.sync_info.on_wait)
    nop.ins.sync_info = mybir.SyncInfo(on_wait=waits[:1], on_update=[])
    for w in waits[1:]:
        extra = self.nc.sync.nop(nofuse=True, hint="tile_tail_wait")
        extra.ins.sync_info = mybir.SyncInfo(on_wait=[w], on_update=[])
    self.nc.sync.drain()
    self.nc.all_engine_barrier()
    assert self.sems is not None
    popped = self.nc._tile_sem_poison_stack.pop()
    assert popped is self._sem_poison
    self.nc.clear_and_free_semaphores(list(self.sems.allocated().values()))
    self.nc.all_engine_barrier()


def build_program(n_cores=N_CORES, b_global=B, fast=True, sim_clean=False):
    """Build the per-core Bass program. fast=True assumes g1>0 and g2>0
    elementwise (true for the reference init: ones), allowing pair-max to
    commute with the BN affine maps. sim_clean adds memsets so CoreSim sees
    no uninitialized reads (junk lanes 20-31 are never consumed)."""
    TileContext._drain_and_barrier = _patched_drain_and_barrier
    AF = mybir.ActivationFunctionType
    OP = mybir.AluOpType
    X = mybir.AxisListType.X
    XY = mybir.AxisListType.XY
    bb = b_global // n_cores
    ni = bb * L
    nc = bacc.Bacc(None, target_bir_lowering=False, num_devices=n_cores)

    # ---- I/O ----
    emb_s_d = nc.declare_dram_parameter("emb_src", [VOCAB, E], BF16, isOutput=False)
    emb_t_d = nc.declare_dram_parameter("emb_trg", [VOCAB, E], BF16, isOutput=False)
    idx_s_d = nc.declare_dram_parameter("idx_src", [128, ni // 16], I16, isOutput=False)
    idx_t_d = nc.declare_dram_parameter("idx_trg", [128, ni // 16], I16, isOutput=False)
    w1t_d = nc.declare_dram_parameter("w1t", [128, 4, 20], BF16, isOutput=False)
    # consts cols: 0=g1 (rows 0-19 & 32-51), 1=beta1 (same rows),
    # 2=g2 (rows 0-39), 3=beta2 (rows 0-39), 4=bfc (all), 5=eps (all),
    # 6=g2 replicated x3 (rows 0-119), 7=beta2 replicated x3
    consts_d = nc.declare_dram_parameter("consts", [128, 8], F32, isOutput=False)
    lhsuv_d = nc.declare_dram_parameter("lhsuv", [60, 80], F32, isOutput=False)
    wfc_d = nc.declare_dram_parameter("wfc_s", [120, PP * 4], F32, isOutput=False)
    out_d = nc.declare_dram_parameter("out", [bb, 1], F32, isOutput=True)

    cc_in_d = nc.dram_tensor("cc_in_d", [40, 5], F32)
    cc_out_d = nc.dram_tensor("cc_out_d", [40, 5], F32, addr_space="Shared")

    inv_l = 1.0 / L
    inv1 = 1.0 / float(b_global * YD)
    inv2 = 1.0 / float(b_global * YD * YD)

    with TileContext(nc) as tc:
        with (
            tc.tile_pool(name="const", bufs=1) as cpool,
            tc.tile_pool(name="work", bufs=1) as wpool,
            tc.tile_pool(name="gath", bufs=2) as gpool,
        ):
            # ---- constant loads ----
            idx_s = cpool.tile([128, ni // 16], I16)
            nc.sync.dma_start(out=idx_s[:], in_=idx_s_d[:])
            idx_t = cpool.tile([128, ni // 16], I16)
            nc.sync.dma_start(out=idx_t[:], in_=idx_t_d[:])
            w1t = cpool.tile([128, 4, 20], BF16)
            nc.sync.dma_start(out=w1t[:], in_=w1t_d[:])
            consts = cpool.tile([128, 8], F32)
            nc.sync.dma_start(out=consts[:], in_=consts_d[:])
            lhsuv = cpool.tile([60, 80], F32)
            nc.sync.dma_start(out=lhsuv[:], in_=lhsuv_d[:])
            wfc = cpool.tile([120, PP * 4], F32)
            nc.sync.dma_start(out=wfc[:], in_=wfc_d[:])

            g1c = consts[0:52, 0:1]
            beta1c = consts[0:52, 1:2]
            g2c120 = consts[0:120, 6:7]
            beta2c120 = consts[0:120, 7:8]

            # ---- stage 1: gather + conv1 -> fT [52, (b,l)] ----
            fsb = wpool.tile([52, ni], F32)
            if sim_clean:
                nc.gpsimd.memset(fsb[:, :], 0.0)
            sides = [(emb_s_d, idx_s, 0), (emb_t_d, idx_t, 32)]
            with tc.tile_pool(name="ps1", bufs=2, space="PSUM") as ps1:
                c0 = 0
                while c0 < ni:
                    cw = min(512, ni - c0)
                    for emb_d, idx, base in sides:
                        xt = gpool.tile([128, 4, cw], BF16, tag=f"xt{base}_{cw}")
                        nc.gpsimd.dma_gather(
                            out_ap=xt[:],
                            in_ap=emb_d[:],
                            idxs_ap=idx[:, c0 // 16:(c0 + cw) // 16],
                            num_idxs=cw,
                            num_idxs_reg=cw,
                            elem_size=E,
                            transpose=True,
                        )
                        pt = ps1.tile([20, cw], F32, space="PSUM", tag=f"pt{base}")
                        for k in range(4):
                            nc.tensor.matmul(
                                out=pt[:],
                                lhsT=w1t[:, k, :],
                                rhs=xt[:, k, :],
                                start=(k == 0),
                                stop=(k == 3),
                            )
                        nc.scalar.activation(
                            out=fsb[base:base + 20, c0:c0 + cw],
                            in_=pt[:],
                            func=AF.Copy,
                        )
                    c0 += cw

            # ---- BN1 stats per (c, b) over l ----
            f3 = fsb[:].rearrange("p (b l) -> p b l", l=L)
            sum_f = wpool.tile([52, bb], F32)
            nc.vector.tensor_reduce(out=sum_f[:], in_=f3, axis=X, op=OP.add)
            fsq = wpool.tile([52, ni], F32)
            nc.scalar.activation(out=fsq[:], in_=fsb[:], func=AF.Square)
            sumsq = wpool.tile([52, bb], F32)
            nc.vector.tensor_reduce(
                out=sumsq[:], in_=fsq[:].rearrange("p (b l) -> p b l", l=L),
                axis=X, op=OP.add,
            )
            mu = wpool.tile([52, bb], F32)
            nc.vector.tensor_scalar_mul(mu[:], sum_f[:], inv_l)
            ex2 = wpool.tile([52, bb], F32)
            nc.vector.tensor_scalar_mul(ex2[:], sumsq[:], inv_l)
            var = wpool.tile([52, bb], F32)
            nc.vector.tensor_tensor(out=var[:], in0=mu[:], in1=mu[:], op=OP.mult)
            nc.vector.tensor_tensor(out=var[:], in0=ex2[:], in1=var[:], op=OP.subtract)
            sd = wpool.tile([52, bb], F32)
            nc.scalar.activation(out=sd[:], in_=var[:], func=AF.Sqrt, bias=consts[0:52, 5:6])
            rs = wpool.tile([52, bb], F32)
            nc.vector.reciprocal(out=rs[:], in_=sd[:])
            va = wpool.tile([52, bb], F32)  # A = rs*g1
            nc.vector.tensor_scalar(out=va[:], in0=rs[:], scalar1=g1c, scalar2=None, op0=OP.mult)

            # ---- pair-max + affine (+relu via ACT bias=beta1) -> st ----
            st = wpool.tile([52, bb * LP], F32)
            if fast:
                # st = relu((maxpair(f) - mu)*A + beta1); pairmax commutes
                # with the increasing affine map (A>0).
                maxf = wpool.tile([52, bb * LP], F32)
                nc.vector.tensor_reduce(
                    out=maxf[:],
                    in_=fsb[:].rearrange("p (b i j) -> p b i j", i=LP, j=2),
                    axis=X, op=OP.max,
                )
                a_b = va[:].rearrange("p (b one) -> p b one", one=1).broadcast_to([52, bb, LP])
                mu_b = mu[:].rearrange("p (b one) -> p b one", one=1).broadcast_to([52, bb, LP])
                m3 = maxf[:].rearrange("p (b i) -> p b i", i=LP)
                nc.vector.tensor_tensor(out=m3, in0=m3, in1=mu_b, op=OP.subtract)
                nc.vector.tensor_tensor(out=m3, in0=m3, in1=a_b, op=OP.mult)
                nc.scalar.activation(out=st[:], in_=maxf[:], func=AF.Relu, bias=beta1c)
            else:
                a_b = va[:].rearrange("p (b one) -> p b one", one=1).broadcast_to([52, bb, L])
                mu_b = mu[:].rearrange("p (b one) -> p b one", one=1).broadcast_to([52, bb, L])
                fb3 = fsb[:].rearrange("p (b l) -> p b l", l=L)
                nc.vector.tensor_tensor(out=fb3, in0=fb3, in1=mu_b, op=OP.subtract)
                nc.vector.tensor_tensor(out=fb3, in0=fb3, in1=a_b, op=OP.mult)
                nc.scalar.activation(out=fsb[:], in_=fsb[:], func=AF.Relu, bias=beta1c)
                nc.vector.tensor_reduce(
                    out=st[:],
                    in_=fsb[:].rearrange("p (b i j) -> p b i j", i=LP, j=2),
                    axis=X, op=OP.max,
                )

            # ---- patches + 1-D convs: U (trg rows) / V (src rows) ----
            patu = wpool.tile([60, bb * YD], F32)
            patv = wpool.tile([60, bb * YD], F32)
            st3 = st[:].rearrange("p (b i) -> p b i", i=LP)
            for dy in range(3):
                nc.sync.dma_start(
                    out=patu[20 * dy:20 * dy + 20, :].rearrange("p (b y) -> p b y", y=YD),
                    in_=st3[32:52, :, dy:dy + YD],
                )
                nc.sync.dma_start(
                    out=patv[20 * dy:20 * dy + 20, :].rearrange("p (b y) -> p b y", y=YD),
                    in_=st3[0:20, :, dy:dy + YD],
                )
            usb = wpool.tile([40, bb * YD], F32)
            vsb = wpool.tile([40, bb * YD], F32)
            with tc.tile_pool(name="ps2", bufs=1, space="PSUM") as ps2:
                ups = ps2.tile([40, bb * YD], F32, space="PSUM")
                vps = ps2.tile([40, bb * YD], F32, space="PSUM")
                for c0 in range(0, bb * YD, 512):
                    cw = min(512, bb * YD - c0)
                    nc.tensor.matmul(out=ups[:, c0:c0 + cw], lhsT=lhsuv[:, 0:40],
                                     rhs=patu[:, c0:c0 + cw], start=True, stop=True)
                    nc.tensor.matmul(out=vps[:, c0:c0 + cw], lhsT=lhsuv[:, 40:80],
                                     rhs=patv[:, c0:c0 + cw], start=True, stop=True)
                nc.scalar.activation(out=usb[:], in_=ups[:], func=AF.Copy)
                nc.scalar.activation(out=vsb[:], in_=vps[:], func=AF.Copy)

            # ---- BN2 partial stats -> cc_in [40, 5] ----
            cc_in = wpool.tile([40, 5], F32)
            rowu = wpool.tile([40, bb], F32)
            rowv = wpool.tile([40, bb], F32)
            u3 = usb[:].rearrange("p (b y) -> p b y", y=YD)
            v3 = vsb[:].rearrange("p (b y) -> p b y", y=YD)
            nc.vector.tensor_reduce(out=rowu[:], in_=u3, axis=X, op=OP.add)
            nc.vector.tensor_reduce(out=rowv[:], in_=v3, axis=X, op=OP.add)
            nc.vector.tensor_reduce(out=cc_in[:, 0:1], in_=rowu[:], axis=X, op=OP.add)
            nc.vector.tensor_reduce(out=cc_in[:, 1:2], in_=rowv[:], axis=X, op=OP.add)
            sqscr = wpool.tile([40, bb * YD], F32)
            nc.scalar.activation(out=sqscr[:], in_=usb[:], func=AF.Square, accum_out=cc_in[:, 2:3])
            nc.scalar.activation(out=sqscr[:], in_=vsb[:], func=AF.Square, accum_out=cc_in[:, 3:4])
            pruv = wpool.tile([40, bb], F32)
            nc.vector.tensor_tensor(out=pruv[:], in0=rowu[:], in1=rowv[:], op=OP.mult)
            nc.vector.tensor_reduce(out=cc_in[:, 4:5], in_=pruv[:], axis=X, op=OP.add)

            # ---- AllReduce of [40,5] stats ----
            nc.sync.dma_start(out=cc_in_d[:], in_=cc_in[:])
            nc.gpsimd.collective_compute(
                "AllReduce", OP.add,
                replica_groups=[list(range(n_cores))],
                ins=[cc_in_d[:]], outs=[cc_out_d[:]],
            )

            # ---- pre-collective tail prep: pair-maxes + G_raw ----
            maxps = wpool.tile([120, bb * PP], F32)
            maxq = wpool.tile([40, bb * PP], F32)
            if fast:
                mpin, mqin = u3, v3
            else:
                mpin, mqin = u3, v3  # slow path scales afterwards (see below)
            nc.vector.tensor_reduce(
                out=maxps[0:40, :],
                in_=mpin[:, :, 0:2 * PP].rearrange("p b (i j) -> p b i j", j=2),
                axis=X, op=OP.max,
            )
            nc.vector.tensor_reduce(
                out=maxq[:],
                in_=mqin[:, :, 0:2 * PP].rearrange("p b (i j) -> p b i j", j=2),
                axis=X, op=OP.max,
            )
            nc.sync.dma_start(out=maxps[40:80, :], in_=maxps[0:40, :])
            nc.sync.dma_start(out=maxps[80:120, :], in_=maxps[0:40, :])
            maxqs = wpool.tile([120, bb * 4], F32)
            if sim_clean:
                nc.gpsimd.memset(maxqs[:], 0.0)
            else:
                nc.gpsimd.memset(maxqs[64:120, :], 0.0)  # covers pad col of last group
            mq3 = maxq[:].rearrange("p (b j) -> p b j", j=PP)
            mqs3 = maxqs[:].rearrange("p (b j) -> p b j", j=4)
            for jg in range(3):
                jc = min(4, PP - 4 * jg)
                nc.sync.dma_start(
                    out=mqs3[40 * jg:40 * jg + 40, :, 0:jc],
                    in_=mq3[:, :, 4 * jg:4 * jg + jc],
                )
            g = wpool.tile([120, bb, PP, 4], F32)
            in0 = maxps[:].rearrange("p (b i one) -> p b i one", i=PP, one=1).broadcast_to([120, bb, PP, 4])
            in1 = maxqs[:].rearrange("p (b one j) -> p b one j", one=1, j=4).broadcast_to([120, bb, PP, 4])
            nc.vector.tensor_tensor(out=g[:], in0=in0, in1=in1, op=OP.add)

            # ---- post-collective: finalize BN2 on 120 partitions ----
            cc120 = wpool.tile([120, 5], F32)
            for jg in range(3):
                nc.sync.dma_start(out=cc120[40 * jg:40 * jg + 40, :], in_=cc_out_d[:])
            mu2 = wpool.tile([120, 1], F32)
            nc.vector.tensor_scalar_mul(mu2[:], cc120[:, 0:1], inv1)
            tmp1 = wpool.tile([120, 1], F32)
            nc.vector.tensor_scalar_mul(tmp1[:], cc120[:, 1:2], inv1)
            nc.vector.tensor_tensor(out=mu2[:], in0=mu2[:], in1=tmp1[:], op=OP.add)
            e2 = wpool.tile([120, 1], F32)
            nc.vector.tensor_scalar_mul(e2[:], cc120[:, 2:3], inv1)
            nc.vector.tensor_scalar_mul(tmp1[:], cc120[:, 3:4], inv1)
            nc.vector.tensor_tensor(out=e2[:], in0=e2[:], in1=tmp1[:], op=OP.add)
            nc.vector.tensor_scalar_mul(tmp1[:], cc120[:, 4:5], 2.0 * inv2)
            nc.vector.tensor_tensor(out=e2[:], in0=e2[:], in1=tmp1[:], op=OP.add)
            nc.vector.tensor_tensor(out=tmp1[:], in0=mu2[:], in1=mu2[:], op=OP.mult)
            nc.vector.tensor_tensor(out=e2[:], in0=e2[:], in1=tmp1[:], op=OP.subtract)
            sd2 = wpool.tile([120, 1], F32)
            nc.scalar.activation(out=sd2[:], in_=e2[:], func=AF.Sqrt, bias=consts[0:120, 5:6])
            rs2 = wpool.tile([120, 1], F32)
            nc.vector.reciprocal(out=rs2[:], in_=sd2[:])
            scale2 = wpool.tile([120, 1], F32)
            nc.vector.tensor_scalar(out=scale2[:], in0=rs2[:], scalar1=g2c120, scalar2=None, op0=OP.mult)
            shq = wpool.tile([120, 1], F32)
            nc.vector.tensor_tensor(out=shq[:], in0=mu2[:], in1=scale2[:], op=OP.mult)
            nc.vector.tensor_scalar(out=shq[:], in0=shq[:], scalar1=-1.0, scalar2=beta2c120, op0=OP.mult, op1=OP.add)

            # ---- G = relu(G_raw*scale + shift) * wfc; weighted reduce ----
            with tc.tile_pool(name="ps3", bufs=1, space="PSUM") as ps3:
                nc.vector.tensor_scalar(out=g[:], in0=g[:], scalar1=scale2[:], scalar2=shq[:], op0=OP.mult, op1=OP.add)
                nc.scalar.activation(out=g[:], in_=g[:], func=AF.Relu)
                wb = wfc[:].rearrange("p (one i j) -> p one i j", one=1, i=PP, j=4).broadcast_to([120, bb, PP, 4])
                nc.vector.tensor_tensor(out=g[:], in0=g[:], in1=wb, op=OP.mult)
                s_t = wpool.tile([120, bb], F32)
                nc.vector.tensor_reduce(out=s_t[:], in_=g[:], axis=XY, op=OP.add)
                ones = wpool.tile([120, 1], F32)
                nc.vector.memset(ones[:], 1.0)
                lps = ps3.tile([bb, 1], F32, space="PSUM")
                nc.tensor.matmul(out=lps[:], lhsT=s_t[:], rhs=ones[:], start=True, stop=True)
                osb = wpool.tile([bb, 1], F32)
                nc.scalar.activation(out=osb[:], in_=lps[:], func=AF.Sigmoid, bias=consts[0:bb, 4:5])
                nc.sync.dma_start(out=out_d[:], in_=osb[:])

    nc.finalize()
    return nc


def _prep_inputs(src_tokens, trg_tokens, emb_src, emb_trg, W1, b1, g1, beta1,
                 W2, b2, g2, beta2, Wfc1, bfc1, Wfc2, bfc2, n_cores=N_CORES):
    """Host-side preprocessing -> per-core in_maps. b1/b2 are dropped: both
    are additive shifts that cancel inside their batch-norms."""
    b_global = src_tokens.shape[0]
    bb = b_global // n_cores
    ni = bb * L

    emb_s_bf = np.asarray(emb_src, np.float32).astype(ml_dtypes.bfloat16)
    emb_t_bf = np.asarray(emb_trg, np.float32).astype(ml_dtypes.bfloat16)

    W1 = np.asarray(W1, np.float32)
    w1t = np.ascontiguousarray(
        W1.T.reshape(4, 128, 20).transpose(1, 0, 2)
    ).astype(ml_dtypes.bfloat16)  # [128, 4, 20]

    W2 = np.asarray(W2, np.float32)
    wrow = W2.sum(axis=3)  # [40, 20, 3] (o, c, dy)
    wcol = W2.sum(axis=2)  # [40, 20, 3] (o, c, dx)
    lhsuv = np.zeros((60, 80), np.float32)
    for dy in range(3):
        lhsuv[dy * 20:(dy + 1) * 20, 0:40] = wrow[:, :, dy].T
        lhsuv[dy * 20:(dy + 1) * 20, 40:80] = wcol[:, :, dy].T

    g1 = np.asarray(g1, np.float32)
    beta1 = np.asarray(beta1, np.float32)
    g2 = np.asarray(g2, np.float32)
    beta2 = np.asarray(beta2, np.float32)
    consts = np.zeros((128, 8), np.float32)
    consts[0:20, 0] = g1
    consts[32:52, 0] = g1
    consts[0:20, 1] = beta1
    consts[32:52, 1] = beta1
    consts[0:40, 2] = g2
    consts[0:40, 3] = beta2
    consts[:, 5] = EPS
    for jg in range(3):
        consts[40 * jg:40 * jg + 40, 6] = g2
        consts[40 * jg:40 * jg + 40, 7] = beta2

    wfc_full = (np.asarray(Wfc2, np.float32) @ np.asarray(Wfc1, np.float32)).reshape(40, PP, PP)
    bfc = float((np.asarray(Wfc2, np.float32) @ np.asarray(bfc1, np.float32)
                 + np.asarray(bfc2, np.float32)).reshape(-1)[0])
    consts[:, 4] = bfc
    wfc_s = np.zeros((120, PP * 4), np.float32)
    for jg in range(3):
        jc = min(4, PP - 4 * jg)
        blk = np.zeros((40, PP, 4), np.float32)
        blk[:, :, 0:jc] = wfc_full[:, :, 4 * jg:4 * jg + jc]
        wfc_s[40 * jg:40 * jg + 40, :] = blk.reshape(40, PP * 4)

    def mk_idx(tok_shard):
        flat = np.asarray(tok_shard, np.int64).reshape(-1)
        assert flat.max() < 32768
        arr = flat.astype(np.int16).reshape(ni // 16, 16).T  # [16, ni/16]
        return np.tile(arr, (8, 1))  # [128, ni/16]

    in_maps = []
    for c in range(n_cores):
        sl = slice(c * bb, (c + 1) * bb)
        in_maps.append({
            "emb_src": emb_s_bf,
            "emb_trg": emb_t_bf,
            "idx_src": mk_idx(src_tokens[sl]),
            "idx_trg": mk_idx(trg_tokens[sl]),
            "w1t": w1t,
            "consts": consts,
            "lhsuv": lhsuv,
            "wfc_s": wfc_s,
        })
    return in_maps, bfc


def _get_executor(nc, n_cores):
    """Compile once and cache a sharded executor for `nc` (the stock
    run_bass_kernel_spmd path retraces+recompiles on every call)."""
    from concourse import bass2jax
    from jax.sharding import Mesh, PartitionSpec
    from jax.experimental.shard_map import shard_map

    bass2jax.install_neuronx_cc_hook()
    partition_name = nc.partition_id_tensor.name if nc.partition_id_tensor else None
    in_names, out_names, out_avals, zero_outs = [], [], [], []
    for alloc in nc.m.functions[0].allocations:
        if not isinstance(alloc, mybir.MemoryLocationSet):
            continue
        name = alloc.memorylocations[0].name
        if alloc.kind == "ExternalInput":
            if name != partition_name:
                in_names.append(name)
        elif alloc.kind == "ExternalOutput":
            shape = tuple(alloc.tensor_shape)
            dtype = mybir.dt.np(alloc.dtype)
            out_names.append(name)
            out_avals.append(jax.core.ShapedArray(shape, dtype))
            zero_outs.append(np.zeros(shape, dtype))
    n_params = len(in_names)
    n_outs = len(out_avals)
    all_in_names = list(in_names) + list(out_names)
    if partition_name is not None:
        all_in_names.append(partition_name)

    def _body(*args):
        operands = list(args)
        if partition_name is not None:
            operands.append(bass2jax.partition_id_tensor())
        outs = bass2jax._bass_exec_p.bind(
            *operands,
            out_avals=tuple(out_avals),
            in_names=tuple(all_in_names),
            out_names=tuple(out_names),
            lowering_input_output_aliases=(),
            sim_require_finite=True,
            sim_require_nnan=True,
            nc=nc,
        )
        return tuple(outs)

    devices = jax.devices()[:n_cores]
    mesh = Mesh(np.asarray(devices), ("core",))
    in_specs = (PartitionSpec("core"),) * (n_params + n_outs)
    out_specs = (PartitionSpec("core"),) * n_outs
    sharded = jax.jit(
        shard_map(_body, mesh=mesh, in_specs=in_specs, out_specs=out_specs,
                  check_rep=False),
        keep_unused=True,
    )
    return sharded, in_names, out_names, zero_outs


def run(nc, in_maps, n_cores=N_CORES):
    key = ("exec", id(nc))
    if key not in _CACHE:
        _CACHE[key] = _get_executor(nc, n_cores)
    sharded, in_names, out_names, zero_outs = _CACHE[key]
    concat_in = [
        np.concatenate([np.asarray(in_maps[c][n]) for c in range(n_cores)], axis=0)
        for n in in_names
    ]
    concat_zeros = [
        np.zeros((n_cores * z.shape[0], *z.shape[1:]), z.dtype) for z in zero_outs
    ]
    out_arrs = sharded(*concat_in, *concat_zeros)
    return {name: np.asarray(out_arrs[i]) for i, name in enumerate(out_names)}


def kernel(src_tokens, trg_tokens, pad_idx, emb_src, emb_trg, W1, b1, g1, beta1,
           W2, b2, g2, beta2, Wfc1, bfc1, Wfc2, bfc2):
    g1a = np.asarray(g1, np.float32)
    g2a = np.asarray(g2, np.float32)
    fast = bool((g1a > 0).all() and (g2a > 0).all())
    if not fast:
        raise NotImplementedError(
            "general-sign g1/g2 path not built; reference init has g=ones"
        )
    key = ("prog", N_CORES, fast)
    if key not in _CACHE:
        _CACHE[key] = build_program(N_CORES, B, fast=fast)
    nc = _CACHE[key]
    in_maps, _ = _prep_inputs(
        src_tokens, trg_tokens, emb_src, emb_trg, W1, b1, g1, beta1,
        W2, b2, g2, beta2, Wfc1, bfc1, Wfc2, bfc2, N_CORES,
    )
    outs = run(nc, in_maps, N_CORES)
    out = outs["out"].reshape(B, 1)
    return out.astype(np.float32)

